# revision 24
# baseline (speedup 1.0000x reference)
"""Trainium2 Bass kernel for nn_MixtureOfExperts (argmax-routed SwiGLU MoE).

Strategy (expert-parallel across 8 NeuronCores):
  - Host computes router logits (fp64, tiny) and the argmax expert per
    token.  Top-2 logit gaps are >=1.7e-4 while fp32 rounding noise is
    ~1e-6, so routing is insensitive to arithmetic order.
  - Tokens are grouped by selected expert; core e gets expert e's tokens
    (padded to a common capacity C) plus that expert's banks and computes
        h = silu(x @ gw) * (x @ uw);  y = h @ dw
    (1/E of the reference FLOPs - the reference runs all experts densely).
  - Host scatters per-core outputs back to token positions.

Matmul dtype: bfloat16 (same 1 cycle/row PE stream rate as float32r, but
stationary-operand LDWEIGHTS gets Fast Weight Load - fp32r tiles loaded at
~188ns each and 672 of them saturated the Tensor queue - and DMA halves).
End-to-end bf16 error vs the fp32 reference is ~4e-3 (gate: 2e-2).

mm1 computes h^T tiles [128h, C]; mm2 runs in transposed orientation
    y^T[128d, C] += dw[k][128h, 128d]-as-lhsT @ h^T[k]
so its free dim is the token count (no partition-padding on the ragged
last token tile) and the host transposes y^T back (cheap numpy).

DMA: the 8 cores share device HBM (~358 GB/s/core fair share) and the
~16MB/core of traffic is roofline-limited during the load phase, so
arrival ORDER is everything:
  - gu is host-packed hs-major ([hs][gate|up][k][128]) so mm1 consumes
    weight columns in strict prefix order;
  - every load is issued in consumption priority order, 3-way
    column-split across the sync/scalar/gpsimd issue queues, so all
    three DMA queues deliver the current prefix in lockstep;
  - x/gu/dw are packed so every DMA descriptor run is >=1.3KB
    (576B-run x transfers previously crawled at ~40 GB/s);
  - dw (needed only by mm2, >60us in) loads strictly after gu.
All weights stay SBUF-resident (~14MB < 24MB SBUF); mm1 for the second
token chunk reuses them with zero extra traffic.
"""

import ml_dtypes
import numpy as np

import concourse.mybir as mybir
import concourse.tile as tile
from concourse import bacc
from concourse.bass_utils import run_bass_kernel_spmd

B, T, D, E, H = 4, 1024, 1024, 8, 2048
BT = B * T
NCORES = 8
P = 128
KD = D // P   # k-tiles for mm1 (contraction over D)
KH = H // P   # k-tiles for mm2 (contraction over H); also # of h^T tiles
ND = D // P   # output d-tiles for mm2 (partition dim of y^T)
F32 = mybir.dt.float32
BF16 = mybir.dt.bfloat16
BF16_NP = ml_dtypes.bfloat16

# "fp32" : exact fp32 matmuls (4 cycles/row on PE)
# "bf16" : bf16 operands (1 cycle/row, fast weight loads, half DMA)
MM_MODE = "bf16"

HS_COLS = 2 * KD * P          # SBUF columns per hs block (2048)

_BUILD_CACHE = {}


def _balanced_chunks(total, step):
    """Split `total` into <=step chunks, as equal as possible (32-aligned)."""
    n = -(-total // step)
    base = -(-total // (n * 32)) * 32
    out = []
    o = 0
    while o < total:
        sz = min(base, total - o)
        out.append((o, sz))
        o += sz
    return out


def _build(C, mm_mode):
    """Build the per-core SPMD Bass kernel for token capacity C."""
    n_chunks = _balanced_chunks(C, 512)   # token tiles in the free dim

    mdt = BF16 if mm_mode == "bf16" else F32

    nc = bacc.Bacc("TRN2", target_bir_lowering=False, debug=False)
    # x^T, chunk-blocked then k-blocked: chunk ni at cols KD*n0, inside it
    # k-block k at cols [k*nn:(k+1)*nn] (contiguous per partition).
    xt = nc.dram_tensor("xt", [P, KD * C], mdt, kind="ExternalInput")
    # gate/up packed hs-major: [hs][gate k0..k7 | up k0..k7], 128 cols each
    gu = nc.dram_tensor("gu", [P, 2 * KD * H], mdt, kind="ExternalInput")
    # down-proj k-blocked: block k = dw[k*128:(k+1)*128, :] at cols [k*D:...]
    dw = nc.dram_tensor("dw", [P, KH * D], mdt, kind="ExternalInput")
    # transposed output y^T
    yt = nc.dram_tensor("yt", [D, C], F32, kind="ExternalOutput")

    with tile.TileContext(nc) as tc:
        with (
            tc.tile_pool(name="xp", bufs=2) as xp,
            tc.tile_pool(name="wg", bufs=1) as wg,
            tc.tile_pool(name="w2", bufs=1) as w2,
            tc.tile_pool(name="hp", bufs=KH) as hp,
            tc.tile_pool(name="outp", bufs=4) as outp,
            tc.tile_pool(name="ps", bufs=8, space="PSUM") as ps,
        ):
            # Input loads round-robin over sync+gpsimd ONLY: a DMA issue on
            # the Scalar queue can block on a recycled-semaphore wait, and
            # everything queued behind it - including the SILU activation
            # TABLE LOAD - then slips by ~20us, exhausting PSUM (silu frees
            # the banks) and stalling the PE.
            load_engines = [nc.sync, nc.gpsimd]
            _rr = [0]

            def load_cols(dst, src, c0, c1):
                """One DMA of a column range, round-robin across the load
                queues.  Loads are emitted in strict consumption priority
                order, so round-robin keeps the hardware DMA rings
                delivering the current prefix of the stream."""
                load_engines[_rr[0] % len(load_engines)].dma_start(
                    dst[:, c0:c1], src[:, c0:c1])
                _rr[0] += 1

            # resident tiles
            x_t = [xp.tile([P, KD * nn], mdt, tag="x", name=f"x{i}")
                   for i, (n0, nn) in enumerate(n_chunks)]
            wgt = wg.tile([P, 2 * KD * H], mdt, tag="w", name="wgt")
            dwt = w2.tile([P, KH * D], mdt, tag="dw", name="dwt")
            h_t = [hp.tile([P, C], mdt, tag="h", name=f"h{k}")
                   for k in range(KH)]

            # ---- loads, in strict consumption-priority order ----
            # Head: x chunk 0 and weight block hs0 in k-pair pieces,
            # interleaved, so the first matmuls arm as soon as ~0.3MB
            # lands; then per-hs weight blocks (each feeds ~3.8us of
            # matmul at ~137 GB/s demand vs ~330 GB/s supply).
            def load_x_piece(ni, k0, k1):
                n0, nn = n_chunks[ni]
                xsrc = xt[:, KD * n0:KD * (n0 + nn)]
                load_cols(x_t[ni][:, :], xsrc, k0 * nn, k1 * nn)

            def load_w_piece(hs, c0, c1):
                load_cols(wgt[:, :], gu[:, :],
                          hs * HS_COLS + c0, hs * HS_COLS + c1)

            for j in range(4):
                load_x_piece(0, 2 * j, 2 * j + 2)
                load_w_piece(0, j * 512, (j + 1) * 512)
            for ni in range(1, len(n_chunks)):
                load_x_piece(ni, 0, KD)
            for half in range(2):
                load_w_piece(1, half * 1024, (half + 1) * 1024)
            for half in range(2):
                load_w_piece(2, half * 1024, (half + 1) * 1024)
            for hs in range(3, KH):
                load_w_piece(hs, 0, HS_COLS)
            for o in range(0, KH * D, KH * D // 2):
                load_cols(dwt[:, :], dw[:, :], o, o + KH * D // 2)

            def w_slice(k, which, hs):
                # k-outer inside each hs block so weight consumption is a
                # strict column-prefix scan (matches the DMA arrival order)
                off = hs * HS_COLS + k * 2 * P + which * P
                return wgt[:, off:off + P]

            def dw_slice(k, dt):
                return dwt[:, k * D + dt * P:k * D + dt * P + P]

            def x_slice(k, ni):
                nn = n_chunks[ni][1]
                return x_t[ni][:, k * nn:(k + 1) * nn]

            def mm1_hs(hs):
                # Both token chunks back-to-back under ONE weight block:
                # each 0.525MB hs block feeds 2x the matmul work, halving
                # the weight-stream bandwidth demand (the HBM is shared by
                # all 8 cores and is the roofline during the load phase).
                ht = h_t[hs]
                for ni, (n0, nn) in enumerate(n_chunks):
                    pa = ps.tile([P, 512], F32, tag="ps", name="pa")[:, :nn]
                    pu = ps.tile([P, 512], F32, tag="ps", name="pu")[:, :nn]
                    # interleave the gate/up accumulation chains: back-to-
                    # back accumulates into one PSUM bank stall the PE
                    for k in range(KD):
                        nc.tensor.matmul(
                            pa[:, :], lhsT=w_slice(k, 0, hs),
                            rhs=x_slice(k, ni),
                            start=(k == 0), stop=(k == KD - 1),
                        )
                        nc.tensor.matmul(
                            pu[:, :], lhsT=w_slice(k, 1, hs),
                            rhs=x_slice(k, ni),
                            start=(k == 0), stop=(k == KD - 1),
                        )
                    nc.scalar.activation(
                        ht[:, n0:n0 + nn], pa[:, :],
                        mybir.ActivationFunctionType.Silu,
                    )
                    nc.vector.tensor_mul(
                        ht[:, n0:n0 + nn], ht[:, n0:n0 + nn], pu[:, :]
                    )

            def mm2(ni):
                n0, nn = n_chunks[ni]
                for dt in range(ND):
                    py = ps.tile([P, 512], F32, tag="ps", name="py")[:, :nn]
                    for k in range(KH):
                        nc.tensor.matmul(
                            py[:, :], lhsT=dw_slice(k, dt),
                            rhs=h_t[k][:, n0:n0 + nn],
                            start=(k == 0), stop=(k == KH - 1),
                        )
                    ot = outp.tile([P, 512], F32, tag="out", name="ot")[:, :nn]
                    nc.vector.tensor_copy(ot[:, :], py[:, :])
                    # outputs on sync/scalar: by mm2 all silus are done, so
                    # scalar's queue is free
                    (nc.sync if dt % 2 == 0 else nc.scalar).dma_start(
                        yt[dt * P:(dt + 1) * P, n0:n0 + nn], ot[:, :])

            # mm2 strictly after all mm1 so gu loads never compete with dw;
            # outputs drain over mm2's ~25us instead of one tail burst.
            for hs in range(KH):
                mm1_hs(hs)
            for ni in range(len(n_chunks)):
                mm2(ni)

    nc.compile()
    return nc


def _get_kernel(C, mm_mode=None):
    """Build (cached).  Falls back to exact fp32 if the bf16 build fails."""
    mm_mode = mm_mode or MM_MODE
    key = (C, mm_mode)
    if key not in _BUILD_CACHE:
        try:
            _BUILD_CACHE[key] = (_build(C, mm_mode), mm_mode)
        except Exception:
            if mm_mode == "fp32":
                raise
            _BUILD_CACHE[key] = (_build(C, "fp32"), "fp32")
    return _BUILD_CACHE[key]


def _route(xf, gate_w):
    """argmax expert per token, computed in fp64 on host (negligible work)."""
    logits = xf.astype(np.float64) @ np.asarray(gate_w, np.float64).T
    return logits.argmax(axis=1)


def _pack_gu(gw_e, uw_e, ndt):
    """[128, 2*KD*H] hs-major, k-outer: [hs][k][gate 128 | up 128]."""
    g = gw_e.reshape(KD, P, KH, P).transpose(1, 2, 0, 3)   # [P, hs, k, 128]
    u = uw_e.reshape(KD, P, KH, P).transpose(1, 2, 0, 3)
    return np.ascontiguousarray(
        np.stack([g, u], axis=3).reshape(P, 2 * KD * H)).astype(ndt)


def _pack_dw(dw_e, ndt):
    """[128, KH*D]: k-blocked partition-major."""
    return np.ascontiguousarray(
        dw_e.reshape(KH, P, D).transpose(1, 0, 2).reshape(P, KH * D)
    ).astype(ndt)


def _pack_x(xe, chunks):
    """[128, KD*C]: chunk-blocked, k-blocked inside each chunk."""
    parts = []
    for n0, nn in chunks:
        parts.append(
            xe[:, n0:n0 + nn].reshape(KD, P, nn).transpose(1, 0, 2)
            .reshape(P, KD * nn))
    return np.ascontiguousarray(np.concatenate(parts, axis=1))


def kernel(x, gate_w, gate_bank, up_bank, down_bank):
    x = np.asarray(x, np.float32)
    assert x.shape == (B, T, D)

    xf = np.ascontiguousarray(x.reshape(BT, D))
    sel = _route(xf, gate_w)
    idx = [np.nonzero(sel == e)[0] for e in range(E)]
    maxc = max(len(i) for i in idx)
    C = max(P, -(-maxc // 32) * 32)
    chunks = _balanced_chunks(C, 512)

    nc, mode = _get_kernel(C)

    ndt = BF16_NP if mode == "bf16" else np.float32
    gate_bank = np.asarray(gate_bank, np.float32)
    up_bank = np.asarray(up_bank, np.float32)
    down_bank = np.asarray(down_bank, np.float32)

    in_maps = []
    for e in range(E):
        xe = np.zeros((D, C), ndt)
        n = len(idx[e])
        if n:
            xe[:, :n] = xf[idx[e]].T.astype(ndt)
        in_maps.append({
            "xt": _pack_x(xe, chunks),
            "gu": _pack_gu(gate_bank[e], up_bank[e], ndt),
            "dw": _pack_dw(down_bank[e], ndt),
        })

    res = run_bass_kernel_spmd(nc, in_maps, core_ids=list(range(NCORES)))

    out = np.empty((BT, D), np.float32)
    for e in range(E):
        n = len(idx[e])
        if n:
            out[idx[e]] = res.results[e]["yt"][:, :n].T
    return out.reshape(B, T, D)


# revision 27
# speedup vs baseline: 1.1863x; 1.1863x over previous
"""Trainium2 Bass kernel for nn_MixtureOfExperts (argmax-routed SwiGLU MoE).

Strategy (expert-parallel across 8 NeuronCores):
  - Host computes router logits (fp64, tiny) and the argmax expert per
    token.  Top-2 logit gaps are >=1.7e-4 while fp32 rounding noise is
    ~1e-6, so routing is insensitive to arithmetic order.
  - Tokens are grouped by selected expert; core e gets expert e's tokens
    (padded to a common capacity C) plus that expert's banks and computes
        h = silu(x @ gw) * (x @ uw);  y = h @ dw
    (1/E of the reference FLOPs - the reference runs all experts densely).
  - Host scatters per-core outputs back to token positions.

Matmul dtype: bfloat16 (same 1 cycle/row PE stream rate as float32r, but
stationary-operand LDWEIGHTS gets Fast Weight Load - fp32r tiles loaded at
~188ns each and 672 of them saturated the Tensor queue - and DMA halves).
End-to-end bf16 error vs the fp32 reference is ~4e-3 (gate: 2e-2).

mm1 computes h^T tiles [128h, C]; mm2 runs in transposed orientation
    y^T[128d, C] += dw[k][128h, 128d]-as-lhsT @ h^T[k]
so its free dim is the token count (no partition-padding on the ragged
last token tile) and the host transposes y^T back (cheap numpy).

DMA: the 8 cores share device HBM (~358 GB/s/core fair share) and the
~16MB/core of traffic is roofline-limited during the load phase, so
arrival ORDER is everything:
  - gu is host-packed hs-major ([hs][gate|up][k][128]) so mm1 consumes
    weight columns in strict prefix order;
  - every load is issued in consumption priority order, 3-way
    column-split across the sync/scalar/gpsimd issue queues, so all
    three DMA queues deliver the current prefix in lockstep;
  - x/gu/dw are packed so every DMA descriptor run is >=1.3KB
    (576B-run x transfers previously crawled at ~40 GB/s);
  - dw (needed only by mm2, >60us in) loads strictly after gu.
All weights stay SBUF-resident (~14MB < 24MB SBUF); mm1 for the second
token chunk reuses them with zero extra traffic.
"""

import ml_dtypes
import numpy as np

import concourse.mybir as mybir
import concourse.tile as tile
from concourse import bacc
from concourse.bass_utils import run_bass_kernel_spmd

B, T, D, E, H = 4, 1024, 1024, 8, 2048
BT = B * T
NCORES = 8
P = 128
KD = D // P   # k-tiles for mm1 (contraction over D)
KH = H // P   # k-tiles for mm2 (contraction over H); also # of h^T tiles
ND = D // P   # output d-tiles for mm2 (partition dim of y^T)
F32 = mybir.dt.float32
BF16 = mybir.dt.bfloat16
BF16_NP = ml_dtypes.bfloat16

# "fp32" : exact fp32 matmuls (4 cycles/row on PE)
# "bf16" : bf16 operands (1 cycle/row, fast weight loads, half DMA)
MM_MODE = "bf16"

HS_COLS = 2 * KD * P          # SBUF columns per hs block (2048)

_BUILD_CACHE = {}


def _balanced_chunks(total, step):
    """Split `total` into <=step chunks, as equal as possible (32-aligned)."""
    n = -(-total // step)
    base = -(-total // (n * 32)) * 32
    out = []
    o = 0
    while o < total:
        sz = min(base, total - o)
        out.append((o, sz))
        o += sz
    return out


def _build(C, mm_mode):
    """Build the per-core SPMD Bass kernel for token capacity C."""
    n_chunks = _balanced_chunks(C, 512)   # token tiles in the free dim

    mdt = BF16 if mm_mode == "bf16" else F32

    nc = bacc.Bacc("TRN2", target_bir_lowering=False, debug=False)
    # x^T, chunk-blocked then k-blocked: chunk ni at cols KD*n0, inside it
    # k-block k at cols [k*nn:(k+1)*nn] (contiguous per partition).
    xt = nc.dram_tensor("xt", [P, KD * C], mdt, kind="ExternalInput")
    # gate/up packed hs-major: [hs][gate k0..k7 | up k0..k7], 128 cols each
    gu = nc.dram_tensor("gu", [P, 2 * KD * H], mdt, kind="ExternalInput")
    # down-proj k-blocked: block k = dw[k*128:(k+1)*128, :] at cols [k*D:...]
    dw = nc.dram_tensor("dw", [P, KH * D], mdt, kind="ExternalInput")
    # transposed output y^T
    yt = nc.dram_tensor("yt", [D, C], F32, kind="ExternalOutput")

    with tile.TileContext(nc) as tc:
        with (
            tc.tile_pool(name="xp", bufs=2) as xp,
            tc.tile_pool(name="wg", bufs=1) as wg,
            tc.tile_pool(name="w2", bufs=1) as w2,
            tc.tile_pool(name="hp", bufs=KH) as hp,
            tc.tile_pool(name="outp", bufs=4) as outp,
            tc.tile_pool(name="ps", bufs=8, space="PSUM") as ps,
        ):
            # Input loads round-robin over sync+gpsimd ONLY: a DMA issue on
            # the Scalar queue can block on a recycled-semaphore wait, and
            # everything queued behind it - including the SILU activation
            # TABLE LOAD - then slips by ~20us, exhausting PSUM (silu frees
            # the banks) and stalling the PE.
            load_engines = [nc.sync, nc.gpsimd]
            _rr = [0]

            def load_cols(dst, src, c0, c1):
                """One DMA of a column range, round-robin across the load
                queues.  Loads are emitted in strict consumption priority
                order, so round-robin keeps the hardware DMA rings
                delivering the current prefix of the stream."""
                load_engines[_rr[0] % len(load_engines)].dma_start(
                    dst[:, c0:c1], src[:, c0:c1])
                _rr[0] += 1

            # resident tiles
            x_t = [xp.tile([P, KD * nn], mdt, tag="x", name=f"x{i}")
                   for i, (n0, nn) in enumerate(n_chunks)]
            wgt = wg.tile([P, 2 * KD * H], mdt, tag="w", name="wgt")
            dwt = w2.tile([P, KH * D], mdt, tag="dw", name="dwt")
            h_t = [hp.tile([P, C], mdt, tag="h", name=f"h{k}")
                   for k in range(KH)]

            # ---- loads, in strict consumption-priority order ----
            # Head: x chunk 0 and weight block hs0 in k-pair pieces,
            # interleaved, so the first matmuls arm as soon as ~0.3MB
            # lands; then per-hs weight blocks (each feeds ~3.8us of
            # matmul at ~137 GB/s demand vs ~330 GB/s supply).
            def load_x_piece(ni, k0, k1):
                n0, nn = n_chunks[ni]
                xsrc = xt[:, KD * n0:KD * (n0 + nn)]
                load_cols(x_t[ni][:, :], xsrc, k0 * nn, k1 * nn)

            def load_w_piece(hs, c0, c1):
                load_cols(wgt[:, :], gu[:, :],
                          hs * HS_COLS + c0, hs * HS_COLS + c1)

            # hs0 pieces in which-outer consumption order: gate k0-3,
            # up k0-3, gate k4-7, up k4-7
            hs0_pieces = [(0, 512), (1024, 1536), (512, 1024), (1536, 2048)]
            for j in range(4):
                load_x_piece(0, 2 * j, 2 * j + 2)
                load_w_piece(0, *hs0_pieces[j])
            for ni in range(1, len(n_chunks)):
                load_x_piece(ni, 0, KD)
            for half in range(2):
                load_w_piece(1, half * 1024, (half + 1) * 1024)
            for half in range(2):
                load_w_piece(2, half * 1024, (half + 1) * 1024)
            for hs in range(3, KH):
                load_w_piece(hs, 0, HS_COLS)
            for o in range(0, KH * D, KH * D // 2):
                load_cols(dwt[:, :], dw[:, :], o, o + KH * D // 2)

            def w_slice(k, which, hs):
                # which-outer inside each hs block: consecutive LDWEIGHTS
                # (gate k / up k alternating) then read 2KB-apart SBUF
                # regions - adjacent-256B weight tiles (k-outer) cost +25ns
                # per LDWEIGHTS in SBUF subbank conflicts.
                off = hs * HS_COLS + which * KD * P + k * P
                return wgt[:, off:off + P]

            def dw_slice(k, dt):
                return dwt[:, k * D + dt * P:k * D + dt * P + P]

            def x_slice(k, ni):
                nn = n_chunks[ni][1]
                return x_t[ni][:, k * nn:(k + 1) * nn]

            def mm1_hs(hs):
                # Both token chunks back-to-back under ONE weight block:
                # each 0.525MB hs block feeds 2x the matmul work, halving
                # the weight-stream bandwidth demand (the HBM is shared by
                # all 8 cores and is the roofline during the load phase).
                ht = h_t[hs]
                for ni, (n0, nn) in enumerate(n_chunks):
                    pa = ps.tile([P, 512], F32, tag="ps", name="pa")[:, :nn]
                    pu = ps.tile([P, 512], F32, tag="ps", name="pu")[:, :nn]
                    # interleave the gate/up accumulation chains: back-to-
                    # back accumulates into one PSUM bank stall the PE
                    for k in range(KD):
                        nc.tensor.matmul(
                            pa[:, :], lhsT=w_slice(k, 0, hs),
                            rhs=x_slice(k, ni),
                            start=(k == 0), stop=(k == KD - 1),
                        )
                        nc.tensor.matmul(
                            pu[:, :], lhsT=w_slice(k, 1, hs),
                            rhs=x_slice(k, ni),
                            start=(k == 0), stop=(k == KD - 1),
                        )
                    nc.scalar.activation(
                        ht[:, n0:n0 + nn], pa[:, :],
                        mybir.ActivationFunctionType.Silu,
                    )
                    nc.vector.tensor_mul(
                        ht[:, n0:n0 + nn], ht[:, n0:n0 + nn], pu[:, :]
                    )

            def mm2(ni):
                n0, nn = n_chunks[ni]
                for dt in range(ND):
                    py = ps.tile([P, 512], F32, tag="ps", name="py")[:, :nn]
                    for k in range(KH):
                        nc.tensor.matmul(
                            py[:, :], lhsT=dw_slice(k, dt),
                            rhs=h_t[k][:, n0:n0 + nn],
                            start=(k == 0), stop=(k == KH - 1),
                        )
                    ot = outp.tile([P, 512], F32, tag="out", name="ot")[:, :nn]
                    nc.vector.tensor_copy(ot[:, :], py[:, :])
                    # outputs on sync/scalar: by mm2 all silus are done, so
                    # scalar's queue is free
                    (nc.sync if dt % 2 == 0 else nc.scalar).dma_start(
                        yt[dt * P:(dt + 1) * P, n0:n0 + nn], ot[:, :])

            # mm2 strictly after all mm1 so gu loads never compete with dw;
            # outputs drain over mm2's ~25us instead of one tail burst.
            for hs in range(KH):
                mm1_hs(hs)
            for ni in range(len(n_chunks)):
                mm2(ni)

    nc.compile()
    return nc


def _get_kernel(C, mm_mode=None):
    """Build (cached).  Falls back to exact fp32 if the bf16 build fails."""
    mm_mode = mm_mode or MM_MODE
    key = (C, mm_mode)
    if key not in _BUILD_CACHE:
        try:
            _BUILD_CACHE[key] = (_build(C, mm_mode), mm_mode)
        except Exception:
            if mm_mode == "fp32":
                raise
            _BUILD_CACHE[key] = (_build(C, "fp32"), "fp32")
    return _BUILD_CACHE[key]


def _route(xf, gate_w):
    """argmax expert per token, computed in fp64 on host (negligible work)."""
    logits = xf.astype(np.float64) @ np.asarray(gate_w, np.float64).T
    return logits.argmax(axis=1)


def _pack_gu(gw_e, uw_e, ndt):
    """[128, 2*KD*H] hs-major, which-outer: [hs][gate k0..k7 | up k0..k7]."""
    g = gw_e.reshape(KD, P, KH, P).transpose(1, 2, 0, 3)   # [P, hs, k, 128]
    u = uw_e.reshape(KD, P, KH, P).transpose(1, 2, 0, 3)
    return np.ascontiguousarray(
        np.stack([g, u], axis=2).reshape(P, 2 * KD * H)).astype(ndt)


def _pack_dw(dw_e, ndt):
    """[128, KH*D]: k-blocked partition-major."""
    return np.ascontiguousarray(
        dw_e.reshape(KH, P, D).transpose(1, 0, 2).reshape(P, KH * D)
    ).astype(ndt)


def _pack_x(xe, chunks):
    """[128, KD*C]: chunk-blocked, k-blocked inside each chunk."""
    parts = []
    for n0, nn in chunks:
        parts.append(
            xe[:, n0:n0 + nn].reshape(KD, P, nn).transpose(1, 0, 2)
            .reshape(P, KD * nn))
    return np.ascontiguousarray(np.concatenate(parts, axis=1))


def kernel(x, gate_w, gate_bank, up_bank, down_bank):
    x = np.asarray(x, np.float32)
    assert x.shape == (B, T, D)

    xf = np.ascontiguousarray(x.reshape(BT, D))
    sel = _route(xf, gate_w)
    idx = [np.nonzero(sel == e)[0] for e in range(E)]
    maxc = max(len(i) for i in idx)
    C = max(P, -(-maxc // 32) * 32)
    chunks = _balanced_chunks(C, 512)

    nc, mode = _get_kernel(C)

    ndt = BF16_NP if mode == "bf16" else np.float32
    gate_bank = np.asarray(gate_bank, np.float32)
    up_bank = np.asarray(up_bank, np.float32)
    down_bank = np.asarray(down_bank, np.float32)

    in_maps = []
    for e in range(E):
        xe = np.zeros((D, C), ndt)
        n = len(idx[e])
        if n:
            xe[:, :n] = xf[idx[e]].T.astype(ndt)
        in_maps.append({
            "xt": _pack_x(xe, chunks),
            "gu": _pack_gu(gate_bank[e], up_bank[e], ndt),
            "dw": _pack_dw(down_bank[e], ndt),
        })

    res = run_bass_kernel_spmd(nc, in_maps, core_ids=list(range(NCORES)))

    out = np.empty((BT, D), np.float32)
    for e in range(E):
        n = len(idx[e])
        if n:
            out[idx[e]] = res.results[e]["yt"][:, :n].T
    return out.reshape(B, T, D)


# revision 30
# speedup vs baseline: 1.1918x; 1.0047x over previous
"""Trainium2 Bass kernel for nn_MixtureOfExperts (argmax-routed SwiGLU MoE).

Strategy (expert-parallel across 8 NeuronCores):
  - Host computes router logits (fp64, tiny) and the argmax expert per
    token.  Top-2 logit gaps are >=1.7e-4 while fp32 rounding noise is
    ~1e-6, so routing is insensitive to arithmetic order.
  - Tokens are grouped by selected expert; core e gets expert e's tokens
    (padded to a common capacity C) plus that expert's banks and computes
        h = silu(x @ gw) * (x @ uw);  y = h @ dw
    (1/E of the reference FLOPs - the reference runs all experts densely).
  - Host scatters per-core outputs back to token positions.

Matmul dtype: bfloat16 (same 1 cycle/row PE stream rate as float32r, but
stationary-operand LDWEIGHTS gets Fast Weight Load - fp32r tiles loaded at
~188ns each and 672 of them saturated the Tensor queue - and DMA halves).
End-to-end bf16 error vs the fp32 reference is ~4e-3 (gate: 2e-2).

mm1 computes h^T tiles [128h, C]; mm2 runs in transposed orientation
    y^T[128d, C] += dw[k][128h, 128d]-as-lhsT @ h^T[k]
so its free dim is the token count (no partition-padding on the ragged
last token tile) and the host transposes y^T back (cheap numpy).

DMA: the 8 cores share device HBM (~358 GB/s/core fair share) and the
~16MB/core of traffic is roofline-limited during the load phase, so
arrival ORDER is everything:
  - gu is host-packed hs-major ([hs][gate|up][k][128]) so mm1 consumes
    weight columns in strict prefix order;
  - every load is issued in consumption priority order, 3-way
    column-split across the sync/scalar/gpsimd issue queues, so all
    three DMA queues deliver the current prefix in lockstep;
  - x/gu/dw are packed so every DMA descriptor run is >=1.3KB
    (576B-run x transfers previously crawled at ~40 GB/s);
  - dw (needed only by mm2, >60us in) loads strictly after gu.
All weights stay SBUF-resident (~14MB < 24MB SBUF); mm1 for the second
token chunk reuses them with zero extra traffic.
"""

import ml_dtypes
import numpy as np

import concourse.mybir as mybir
import concourse.tile as tile
from concourse import bacc
from concourse.bass_utils import run_bass_kernel_spmd

B, T, D, E, H = 4, 1024, 1024, 8, 2048
BT = B * T
NCORES = 8
P = 128
KD = D // P   # k-tiles for mm1 (contraction over D)
KH = H // P   # k-tiles for mm2 (contraction over H); also # of h^T tiles
ND = D // P   # output d-tiles for mm2 (partition dim of y^T)
F32 = mybir.dt.float32
BF16 = mybir.dt.bfloat16
BF16_NP = ml_dtypes.bfloat16

# "fp32" : exact fp32 matmuls (4 cycles/row on PE)
# "bf16" : bf16 operands (1 cycle/row, fast weight loads, half DMA)
MM_MODE = "bf16"

HS_COLS = 2 * KD * P          # SBUF columns per hs block (2048)

_BUILD_CACHE = {}


def _balanced_chunks(total, step):
    """Split `total` into <=step chunks, as equal as possible (32-aligned)."""
    n = -(-total // step)
    base = -(-total // (n * 32)) * 32
    out = []
    o = 0
    while o < total:
        sz = min(base, total - o)
        out.append((o, sz))
        o += sz
    return out


def _build(C, mm_mode):
    """Build the per-core SPMD Bass kernel for token capacity C."""
    n_chunks = _balanced_chunks(C, 512)   # token tiles in the free dim

    mdt = BF16 if mm_mode == "bf16" else F32

    nc = bacc.Bacc("TRN2", target_bir_lowering=False, debug=False)
    # x^T, chunk-blocked then k-blocked: chunk ni at cols KD*n0, inside it
    # k-block k at cols [k*nn:(k+1)*nn] (contiguous per partition).
    xt = nc.dram_tensor("xt", [P, KD * C], mdt, kind="ExternalInput")
    # gate/up packed hs-major: [hs][gate k0..k7 | up k0..k7], 128 cols each
    gu = nc.dram_tensor("gu", [P, 2 * KD * H], mdt, kind="ExternalInput")
    # down-proj k-blocked: block k = dw[k*128:(k+1)*128, :] at cols [k*D:...]
    dw = nc.dram_tensor("dw", [P, KH * D], mdt, kind="ExternalInput")
    # transposed output y^T
    yt = nc.dram_tensor("yt", [D, C], F32, kind="ExternalOutput")

    with tile.TileContext(nc) as tc:
        with (
            tc.tile_pool(name="xp", bufs=2) as xp,
            tc.tile_pool(name="wg", bufs=1) as wg,
            tc.tile_pool(name="w2", bufs=1) as w2,
            tc.tile_pool(name="hp", bufs=KH) as hp,
            tc.tile_pool(name="outp", bufs=4) as outp,
            tc.tile_pool(name="ps", bufs=8, space="PSUM") as ps,
        ):
            # Input loads round-robin over sync+gpsimd ONLY: a DMA issue on
            # the Scalar queue can block on a recycled-semaphore wait, and
            # everything queued behind it - including the SILU activation
            # TABLE LOAD - then slips by ~20us, exhausting PSUM (silu frees
            # the banks) and stalling the PE.
            load_engines = [nc.sync, nc.gpsimd]
            _rr = [0]

            def load_cols(dst, src, c0, c1):
                """One DMA of a column range, round-robin across the load
                queues.  Loads are emitted in strict consumption priority
                order, so round-robin keeps the hardware DMA rings
                delivering the current prefix of the stream."""
                load_engines[_rr[0] % len(load_engines)].dma_start(
                    dst[:, c0:c1], src[:, c0:c1])
                _rr[0] += 1

            # resident tiles
            x_t = [xp.tile([P, KD * nn], mdt, tag="x", name=f"x{i}")
                   for i, (n0, nn) in enumerate(n_chunks)]
            wgt = wg.tile([P, 2 * KD * H], mdt, tag="w", name="wgt")
            dwt = w2.tile([P, KH * D], mdt, tag="dw", name="dwt")
            h_t = [hp.tile([P, C], mdt, tag="h", name=f"h{k}")
                   for k in range(KH)]

            # PE warm-up: the HAM power manager holds the PE at half clock
            # until it has seen sustained activity; real weights only land
            # at ~9.5us, so burn ~2.5us of dummy matmuls on a zeroed
            # scratch tile to start the HAM activity window early.
            scr = xp.tile([P, P], mdt, tag="scr", name="scr")
            nc.vector.memset(scr[:, :], 0)
            wps = ps.tile([P, 512], F32, tag="ps", name="wps")
            for _ in range(24):
                nc.tensor.matmul(wps[:, :P], lhsT=scr[:, :], rhs=scr[:, :],
                                 start=True, stop=True)

            # ---- loads, in strict consumption-priority order ----
            # Head: x chunk 0 and weight block hs0 in k-pair pieces,
            # interleaved, so the first matmuls arm as soon as ~0.3MB
            # lands; then per-hs weight blocks (each feeds ~3.8us of
            # matmul at ~137 GB/s demand vs ~330 GB/s supply).
            def load_x_piece(ni, k0, k1):
                n0, nn = n_chunks[ni]
                xsrc = xt[:, KD * n0:KD * (n0 + nn)]
                load_cols(x_t[ni][:, :], xsrc, k0 * nn, k1 * nn)

            def load_w_piece(hs, c0, c1):
                load_cols(wgt[:, :], gu[:, :],
                          hs * HS_COLS + c0, hs * HS_COLS + c1)

            # Head: hs0 weights in per-k-pair gate/up pieces interleaved
            # with x k-pair pieces, so the consumption ladder (k0 g, k0 u,
            # k1 g, ... with x block k) is fed in arrival order across
            # both queues.
            GK, UK = 0, KD * P   # gate / up halves of an hs block
            for j in range(4):
                load_x_piece(0, 2 * j, 2 * j + 2)
                load_w_piece(0, GK + j * 256, GK + (j + 1) * 256)
                load_w_piece(0, UK + j * 256, UK + (j + 1) * 256)
            for ni in range(1, len(n_chunks)):
                load_x_piece(ni, 0, KD)
            for half in range(2):
                load_w_piece(1, half * 1024, (half + 1) * 1024)
            for half in range(2):
                load_w_piece(2, half * 1024, (half + 1) * 1024)
            for hs in range(3, KH):
                load_w_piece(hs, 0, HS_COLS)
            for o in range(0, KH * D, KH * D // 2):
                load_cols(dwt[:, :], dw[:, :], o, o + KH * D // 2)

            def w_slice(k, which, hs):
                # which-outer inside each hs block: consecutive LDWEIGHTS
                # (gate k / up k alternating) then read 2KB-apart SBUF
                # regions - adjacent-256B weight tiles (k-outer) cost +25ns
                # per LDWEIGHTS in SBUF subbank conflicts.
                off = hs * HS_COLS + which * KD * P + k * P
                return wgt[:, off:off + P]

            def dw_slice(k, dt):
                return dwt[:, k * D + dt * P:k * D + dt * P + P]

            def x_slice(k, ni):
                nn = n_chunks[ni][1]
                return x_t[ni][:, k * nn:(k + 1) * nn]

            def mm1_hs(hs):
                # Both token chunks back-to-back under ONE weight block:
                # each 0.525MB hs block feeds 2x the matmul work, halving
                # the weight-stream bandwidth demand (the HBM is shared by
                # all 8 cores and is the roofline during the load phase).
                ht = h_t[hs]
                for ni, (n0, nn) in enumerate(n_chunks):
                    pa = ps.tile([P, 512], F32, tag="ps", name="pa")[:, :nn]
                    pu = ps.tile([P, 512], F32, tag="ps", name="pu")[:, :nn]
                    # interleave the gate/up accumulation chains: back-to-
                    # back accumulates into one PSUM bank stall the PE
                    for k in range(KD):
                        nc.tensor.matmul(
                            pa[:, :], lhsT=w_slice(k, 0, hs),
                            rhs=x_slice(k, ni),
                            start=(k == 0), stop=(k == KD - 1),
                        )
                        nc.tensor.matmul(
                            pu[:, :], lhsT=w_slice(k, 1, hs),
                            rhs=x_slice(k, ni),
                            start=(k == 0), stop=(k == KD - 1),
                        )
                    nc.scalar.activation(
                        ht[:, n0:n0 + nn], pa[:, :],
                        mybir.ActivationFunctionType.Silu,
                    )
                    nc.vector.tensor_mul(
                        ht[:, n0:n0 + nn], ht[:, n0:n0 + nn], pu[:, :]
                    )

            def mm2(ni):
                n0, nn = n_chunks[ni]
                for dt in range(ND):
                    py = ps.tile([P, 512], F32, tag="ps", name="py")[:, :nn]
                    for k in range(KH):
                        nc.tensor.matmul(
                            py[:, :], lhsT=dw_slice(k, dt),
                            rhs=h_t[k][:, n0:n0 + nn],
                            start=(k == 0), stop=(k == KH - 1),
                        )
                    ot = outp.tile([P, 512], F32, tag="out", name="ot")[:, :nn]
                    nc.vector.tensor_copy(ot[:, :], py[:, :])
                    # outputs on sync/scalar: by mm2 all silus are done, so
                    # scalar's queue is free
                    (nc.sync if dt % 2 == 0 else nc.scalar).dma_start(
                        yt[dt * P:(dt + 1) * P, n0:n0 + nn], ot[:, :])

            # mm2 strictly after all mm1 so gu loads never compete with dw;
            # outputs drain over mm2's ~25us instead of one tail burst.
            for hs in range(KH):
                mm1_hs(hs)
            for ni in range(len(n_chunks)):
                mm2(ni)

    nc.compile()
    return nc


def _get_kernel(C, mm_mode=None):
    """Build (cached).  Falls back to exact fp32 if the bf16 build fails."""
    mm_mode = mm_mode or MM_MODE
    key = (C, mm_mode)
    if key not in _BUILD_CACHE:
        try:
            _BUILD_CACHE[key] = (_build(C, mm_mode), mm_mode)
        except Exception:
            if mm_mode == "fp32":
                raise
            _BUILD_CACHE[key] = (_build(C, "fp32"), "fp32")
    return _BUILD_CACHE[key]


def _route(xf, gate_w):
    """argmax expert per token, computed in fp64 on host (negligible work)."""
    logits = xf.astype(np.float64) @ np.asarray(gate_w, np.float64).T
    return logits.argmax(axis=1)


def _pack_gu(gw_e, uw_e, ndt):
    """[128, 2*KD*H] hs-major, which-outer: [hs][gate k0..k7 | up k0..k7]."""
    g = gw_e.reshape(KD, P, KH, P).transpose(1, 2, 0, 3)   # [P, hs, k, 128]
    u = uw_e.reshape(KD, P, KH, P).transpose(1, 2, 0, 3)
    return np.ascontiguousarray(
        np.stack([g, u], axis=2).reshape(P, 2 * KD * H)).astype(ndt)


def _pack_dw(dw_e, ndt):
    """[128, KH*D]: k-blocked partition-major."""
    return np.ascontiguousarray(
        dw_e.reshape(KH, P, D).transpose(1, 0, 2).reshape(P, KH * D)
    ).astype(ndt)


def _pack_x(xe, chunks):
    """[128, KD*C]: chunk-blocked, k-blocked inside each chunk."""
    parts = []
    for n0, nn in chunks:
        parts.append(
            xe[:, n0:n0 + nn].reshape(KD, P, nn).transpose(1, 0, 2)
            .reshape(P, KD * nn))
    return np.ascontiguousarray(np.concatenate(parts, axis=1))


def kernel(x, gate_w, gate_bank, up_bank, down_bank):
    x = np.asarray(x, np.float32)
    assert x.shape == (B, T, D)

    xf = np.ascontiguousarray(x.reshape(BT, D))
    sel = _route(xf, gate_w)
    idx = [np.nonzero(sel == e)[0] for e in range(E)]
    maxc = max(len(i) for i in idx)
    C = max(P, -(-maxc // 32) * 32)
    chunks = _balanced_chunks(C, 512)

    nc, mode = _get_kernel(C)

    ndt = BF16_NP if mode == "bf16" else np.float32
    gate_bank = np.asarray(gate_bank, np.float32)
    up_bank = np.asarray(up_bank, np.float32)
    down_bank = np.asarray(down_bank, np.float32)

    in_maps = []
    for e in range(E):
        xe = np.zeros((D, C), ndt)
        n = len(idx[e])
        if n:
            xe[:, :n] = xf[idx[e]].T.astype(ndt)
        in_maps.append({
            "xt": _pack_x(xe, chunks),
            "gu": _pack_gu(gate_bank[e], up_bank[e], ndt),
            "dw": _pack_dw(down_bank[e], ndt),
        })

    res = run_bass_kernel_spmd(nc, in_maps, core_ids=list(range(NCORES)))

    out = np.empty((BT, D), np.float32)
    for e in range(E):
        n = len(idx[e])
        if n:
            out[idx[e]] = res.results[e]["yt"][:, :n].T
    return out.reshape(B, T, D)
